# revision 22
# baseline (speedup 1.0000x reference)
# DiabaticReadout forward on Trainium2 (Bass/Tile), 8-core data-parallel.
#
# Per sample i: H = [[d0, lam], [lam, d1]] (2x2 symmetric).  Eigenvalues in
# closed form:
#   mean    = 0.5*(d0+d1)
#   halfgap = sqrt((0.5*(d0-d1))^2 + lam^2)
#   e0, e1  = mean -/+ halfgap          (ascending, matches eigh)
#
# Purely elementwise and memory-bound, so the dominant lever is HBM bytes.
# The harness tolerance (rel err vs max|out| < 2e-2) leaves ~20x of slack
# over fp16 rounding, so the device streams fp16 end to end; that halves
# DMA traffic vs f32: 10 B/sample instead of 20 -> ~35 us/core at the
# ~358 GB/s per-core HBM ceiling (vs ~70 us for the f32 version).
#
# The host hands the device the equivalent linear reparameterization of H
#   am = 0.5*(d0+d1)   ad = 0.5*(d0-d1)   l = lam
# (H = am*I + [[ad, l], [l, -ad]]), folding the constant basis change into
# the fp16 downcast it has to do anyway.  The eigensolve proper runs on
# device:  e0, e1 = am -/+ sqrt(ad^2 + l^2).
#
# Engine split per tile (all fp16, all contiguous so DVE runs its packed
# 2x_1p mode at 2 elem/cycle/lane -- only plain tensor_tensor has that
# uop, which is why there is no scalar_tensor_tensor here and why the
# outputs are two planar tensors rather than an interleaved [N, 2]):
#   DVE: l2 = l*l, s = d2 + l2, e0 = am - hg, e1 = am + hg   (4 passes)
#   ACT: d2 = Square(ad), hg = Sqrt(s)  (2 passes, one pinned table)
# Per core (1.25M samples): DVE ~26 us, ACT ~21 us, DMA 12.5 MB ~35 us ->
# DMA-bound with slack on both compute engines.  Loads issue on the SP
# HWDGE ring, stores on the GPSIMD SWDGE ring so neither stream queues
# behind the other; the host zips/upcasts the two output planes.
#
# Tile schedule [2914, 2914, 2914, 1024]: with both compute engines well
# under the DMA roofline, fewer/bigger tiles win (fewer DMA-issue
# instructions, semaphore waits, and SWDGE stores); the smaller last tile
# keeps the post-load drain short.  Measured ~46-49 us vs ~86 us for the
# staged f32 kernel; run-to-run spread is +-4 us of inter-core HBM
# contention (8 cores pull ~2.8 TB/s, the device ceiling).

import numpy as np

import concourse.bacc as bacc_mod
import concourse.tile as tile
from concourse import bacc, mybir
from concourse.bass_utils import run_bass_kernel_spmd

import contextlib


@contextlib.contextmanager
def _pin_act_table(keep="sqrt_and_others"):
    """Both our activations (Square, Sqrt) live in the single
    `sqrt_and_others` set, but the table-load pass greedily picks the first
    set containing each function, which alternates tables per tile
    (~2.5us/tile of ACT_TABLE_LOAD thrash).  Present every other set as
    empty during compile so the pass pins everything to one table; indices
    stay aligned with act_info.json."""
    orig = bacc_mod.get_activation_tables

    def patched(arch):
        t = orig(arch)
        assert keep in t, sorted(t)
        return {name: (funcs if name == keep else set()) for name, funcs in t.items()}

    bacc_mod.get_activation_tables = patched
    try:
        yield
    finally:
        bacc_mod.get_activation_tables = orig

N_CORES = 8
P = 128  # SBUF partitions

_cache = {}


def _tile_schedule(rows, f_tile, ramp, ramp_end=()):
    """Tile-size schedule: optional small prologue/epilogue tiles so the
    pipeline fills/drains quickly, f_tile-sized tiles in the middle."""
    head, tail = [], []
    left = rows
    for s in ramp:
        if left <= 0:
            break
        s = min(s, left)
        head.append(s)
        left -= s
    for s in ramp_end:
        if left <= 0:
            break
        s = min(s, left)
        tail.append(s)
        left -= s
    mid = []
    while left > 0:
        s = min(f_tile, left)
        mid.append(s)
        left -= s
    return head + mid + tail[::-1]


@contextlib.contextmanager
def _trim_epilogue(level):
    """Slim down TileContext's exit sequence (drain + barrier + sem-clear +
    barrier).  level 1 drops the trailing all_engine_barrier (the final
    static drains + NEFF completion handshake still serialize the engines
    behind the sem-clear's issuing engine); level 2 also drops the
    sem-clear (safe across re-executions only if the preamble re-inits the
    sem file -- validated by the repeated-run correctness check)."""
    if not level:
        yield
        return
    orig = tile.TileContext._drain_and_barrier

    def patched(self, tick_clock, wait_clock):
        from concourse.vector_clock import ScopedClock

        drain_inst = self.nc.sync.drain()
        wait_clock.add_sem_waits(
            drain_inst.ins, ScopedClock({None: tick_clock.global_clock})
        )
        self.nc.all_engine_barrier()
        assert self.sems is not None
        popped = self.nc._tile_sem_poison_stack.pop()
        assert popped is self._sem_poison
        if level < 2:
            self.nc.clear_and_free_semaphores(
                list(self.sems.allocated().values())
            )

    tile.TileContext._drain_and_barrier = patched
    try:
        yield
    finally:
        tile.TileContext._drain_and_barrier = orig


def _build(rows, f_tile=2914, in_bufs=3, out_bufs=4, tmp_bufs=3,
           store_engine="gpsimd", e1_store_engine=None, l_engine="sync",
           l2_engine="vector", ramp=(), ramp_end=(1024,), epilogue_trim=0,
           pack_amad=False):
    """Per-core Bass module: inputs am,ad,l = [P*rows] fp16, outputs
    e0,e1 = [P*rows] fp16."""
    C = P * rows
    f16 = mybir.dt.float16
    Act = mybir.ActivationFunctionType

    nc = bacc.Bacc(
        "TRN2",
        target_bir_lowering=False,
        debug=False,
        num_devices=N_CORES,
    )
    if pack_amad:
        md = nc.dram_tensor("md", [2 * C], f16, kind="ExternalInput").ap()
        mdv = md.rearrange("(two p f) -> p two f", two=2, p=P)
    else:
        am = nc.dram_tensor("am", [C], f16, kind="ExternalInput").ap()
        ad = nc.dram_tensor("ad", [C], f16, kind="ExternalInput").ap()
        amv = am.rearrange("(p f) -> p f", p=P)
        adv = ad.rearrange("(p f) -> p f", p=P)
    l = nc.dram_tensor("l", [C], f16, kind="ExternalInput").ap()
    e0 = nc.dram_tensor("e0", [C], f16, kind="ExternalOutput").ap()
    e1 = nc.dram_tensor("e1", [C], f16, kind="ExternalOutput").ap()

    lv = l.rearrange("(p f) -> p f", p=P)
    e0v = e0.rearrange("(p f) -> p f", p=P)
    e1v = e1.rearrange("(p f) -> p f", p=P)

    store_eng = getattr(nc, store_engine)
    e1_store_eng = getattr(nc, e1_store_engine or store_engine)
    l_eng = getattr(nc, l_engine)
    l2_eng = getattr(nc, l2_engine)
    sizes = _tile_schedule(rows, f_tile, ramp, ramp_end)
    # Even tile sizes keep every operand's packed-pair count whole for the
    # DVE 2x_1p mode (odd sizes still run correctly, just fractionally
    # slower on the last pair).
    assert sum(sizes) == rows, (sizes, rows)

    with _trim_epilogue(epilogue_trim), tile.TileContext(nc) as tc:
        with (
            tc.tile_pool(name="ins", bufs=in_bufs) as ins,
            tc.tile_pool(name="outs", bufs=out_bufs) as outs,
            tc.tile_pool(name="tmp", bufs=tmp_bufs) as tmp,
        ):
            f0 = 0
            for F in sizes:
                sl = slice(f0, f0 + F)

                # ad feeds the critical path (ad -> d2 -> s -> sqrt); am is
                # only consumed by the final two output ops.
                if pack_amad:
                    t_md = ins.tile([P, 2, F], f16, tag="md")
                    nc.sync.dma_start(t_md[:], mdv[:, :, sl])
                    t_am, t_ad = t_md[:, 0, :], t_md[:, 1, :]
                    t_l = ins.tile([P, F], f16, tag="l")
                    l_eng.dma_start(t_l[:], lv[:, sl])
                    t_am_ap, t_ad_ap = t_am, t_ad
                else:
                    t_ad = ins.tile([P, F], f16, tag="ad")
                    nc.sync.dma_start(t_ad[:], adv[:, sl])
                    t_l = ins.tile([P, F], f16, tag="l")
                    l_eng.dma_start(t_l[:], lv[:, sl])
                    t_am = ins.tile([P, F], f16, tag="am")
                    nc.sync.dma_start(t_am[:], amv[:, sl])
                    t_am_ap, t_ad_ap = t_am[:], t_ad[:]

                t_d2 = tmp.tile([P, F], f16, tag="d2")
                nc.scalar.activation(t_d2[:], t_ad_ap, Act.Square)
                t_l2 = tmp.tile([P, F], f16, tag="l2")
                l2_eng.tensor_mul(t_l2[:], t_l[:], t_l[:])

                t_s = tmp.tile([P, F], f16, tag="s")
                nc.vector.tensor_add(t_s[:], t_d2[:], t_l2[:])
                t_hg = tmp.tile([P, F], f16, tag="hg")
                nc.scalar.activation(t_hg[:], t_s[:], Act.Sqrt)

                t_e0 = outs.tile([P, F], f16, tag="e0")
                nc.vector.tensor_sub(t_e0[:], t_am_ap, t_hg[:])
                t_e1 = outs.tile([P, F], f16, tag="e1")
                nc.vector.tensor_add(t_e1[:], t_am_ap, t_hg[:])
                store_eng.dma_start(e0v[:, sl], t_e0[:])
                e1_store_eng.dma_start(e1v[:, sl], t_e1[:])

                f0 += F
    with _pin_act_table():
        nc.compile()
    return nc


def _get_nc(rows, **cfg):
    for k in ("ramp", "ramp_end"):
        if k in cfg:
            cfg[k] = tuple(cfg[k])
    key = (rows, tuple(sorted(cfg.items())))
    if key not in _cache:
        _cache[key] = _build(rows, **cfg)
    return _cache[key]


def kernel(d0, d1, lam, _trace=False, **cfg):
    d0 = np.asarray(d0)
    d1 = np.asarray(d1)
    lam = np.asarray(lam)
    n = d0.shape[0]

    # Linear basis change folded into the fp16 downcast (in f32, so ad
    # keeps full precision even when d0 ~ d1).
    d0 = d0.ravel().astype(np.float32, copy=False)
    d1 = d1.ravel().astype(np.float32, copy=False)
    am = ((d0 + d1) * np.float32(0.5)).astype(np.float16)
    ad = ((d0 - d1) * np.float32(0.5)).astype(np.float16)
    l = lam.ravel().astype(np.float16)

    # Per-core sample count: multiple of 128, cores cover ceil(n / 8).
    rows = -(-n // (N_CORES * P))  # ceil
    C = P * rows
    total = N_CORES * C
    pad = total - n
    if pad:
        z = np.zeros(pad, np.float16)
        am = np.concatenate([am, z])
        ad = np.concatenate([ad, z])
        l = np.concatenate([l, z])

    if cfg.get("pack_amad"):
        in_maps = [
            {
                "md": np.concatenate(
                    [am[c * C : (c + 1) * C], ad[c * C : (c + 1) * C]]
                ),
                "l": np.ascontiguousarray(l[c * C : (c + 1) * C]),
            }
            for c in range(N_CORES)
        ]
    else:
        in_maps = [
            {
                "am": np.ascontiguousarray(am[c * C : (c + 1) * C]),
                "ad": np.ascontiguousarray(ad[c * C : (c + 1) * C]),
                "l": np.ascontiguousarray(l[c * C : (c + 1) * C]),
            }
            for c in range(N_CORES)
        ]

    nc = _get_nc(rows, **cfg)
    res = run_bass_kernel_spmd(
        nc, in_maps, core_ids=list(range(N_CORES)), trace=_trace
    )
    global last_results
    last_results = res
    out = np.empty((total, 2), np.float32)
    for c in range(N_CORES):
        out[c * C : (c + 1) * C, 0] = res.results[c]["e0"]
        out[c * C : (c + 1) * C, 1] = res.results[c]["e1"]
    return out[:n]


last_results = None
